# revision 22
# baseline (speedup 1.0000x reference)
"""Trainium2 Bass kernel for nn_Attention_Joint_MaxPool.

Math (see reference):
  q = (Wq*scale) @ x                        (B, C, N), heads on rows
  xsr = conv2x2s2(x) ; k = Wk @ BN(xsr)     (B, C, Nk=1024)
  attn = max over keys of q_h . k_h         (B, NH, N)
  s = sum over heads of attn                (B, N)
  out[b,c,n] = (Wproj @ mean_n x)[c] * s[b,n] + bproj[c]

Weight folding done on host:
  g = gamma/sqrt(var+eps); A = Wk * g[None,:]
  k = sum_e (A @ Wsr[:,:,e]) @ x_sub[e] + ck,  ck = A@bsr + Wk@(beta-mean*g)
  pv[b] = Wproj @ mean_n x[b]   (rank-1 output structure)

Max over keys via pair cascade: for key pairs (even, odd):
  max(a,b) = a + relu(b-a); a = q.k_even, (b-a) = q.(k_odd-k_even)
  k_even and k_diff both come from matmuls of host-prearranged x columns,
  relu on ScalarE, the add is a PE identity-matmul accumulate into PSUM,
  then one VectorE reduce_max per head over the 512 pair-maxes.

Sharding: 8 cores; core i -> batch i//2, token half i%2 (2048 tokens).
Each core is fully independent (no collectives).
"""

import os
import sys
import types
import numpy as np

# ---------------------------------------------------------------------------
# problem constants (hardcoded; kernel.py must be self-contained)
# ---------------------------------------------------------------------------
B, C, N = 4, 512, 4096
NH, HD = 8, 64
SR = 2
EPS = 1e-5
HW_ = 64                      # H = W = 64
T = N // 2                    # tokens per core
NK = 1024                     # conv output positions (keys)
NKE = NK // 2                 # even keys (pairs)
MB = C // 128                 # 4 channel blocks
KC = C // 128                 # 4 contraction chunks
NCORES = 8

_cache = {}


# ---------------------------------------------------------------------------
# workarounds for this container's toolchain
# ---------------------------------------------------------------------------
def _install_fixes():
    import concourse.tile as tile
    import concourse.mybir as mybir
    from concourse.vector_clock import ScopedClock

    if getattr(tile.TileContext, "_drain_patched", False):
        return

    def _patched_drain_and_barrier(self, tick_clock, wait_clock):
        nc = self.nc
        probe = nc.sync.nop(nofuse=True, hint="drain_wait_carrier")
        wait_clock.add_sem_waits(
            probe.ins, ScopedClock({None: tick_clock.global_clock})
        )
        waits = list(probe.ins.sync_info.on_wait) if probe.ins.sync_info else []
        if len(waits) > 1:
            probe.ins.sync_info = mybir.SyncInfo(on_wait=waits[:1], on_update=[])
            for w in waits[1:]:
                extra = nc.sync.nop(nofuse=True, hint="drain_wait_carrier")
                extra.ins.sync_info = mybir.SyncInfo(on_wait=[w], on_update=[])
        nc.sync.drain()
        nc.all_engine_barrier()
        assert self.sems is not None
        popped = nc._tile_sem_poison_stack.pop()
        assert popped is self._sem_poison
        nc.clear_and_free_semaphores(list(self.sems.allocated().values()))
        nc.all_engine_barrier()

    tile.TileContext._drain_and_barrier = _patched_drain_and_barrier
    tile.TileContext._drain_patched = True


def _split_multi_waits(nc):
    """This walrus build allows only one sync-wait per instruction; hoist
    extra waits onto same-engine nops inserted just before the instruction."""
    import concourse.mybir as mybir

    ctr = 0
    for f in nc.m.functions:
        for bb in f.blocks:
            changed = False
            out = []
            for inst in bb.instructions:
                si = inst.sync_info
                tname = type(inst).__name__
                if (si is not None and si.on_wait and len(si.on_wait) > 1
                        and "Collective" not in tname):
                    waits = list(si.on_wait)
                    for w in waits[:-1]:
                        ctr += 1
                        nop = mybir.InstNoOp(
                            name=f"I-ws-{ctr}",
                            engine=inst.engine,
                            sync_info=mybir.SyncInfo(on_wait=[w], on_update=[]),
                        )
                        nc.register_instruction(nop, overwrite=True)
                        out.append(nop)
                    inst.sync_info = mybir.SyncInfo(
                        on_wait=waits[-1:], on_update=list(si.on_update)
                    )
                    changed = True
                out.append(inst)
            if changed:
                bb.instructions = out


def _install_ntff_hook():
    """Provide antenv.axon_hooks (missing in this image) so trace=True works."""
    try:
        from antenv import axon_hooks  # noqa: F401
        return
    except ImportError:
        pass
    try:
        import antenv
        from trn_agent_boot.trn_boot import _ntff_profile_via_ctypes
    except ImportError:
        return
    mod = types.ModuleType("antenv.axon_hooks")
    _hook = [None]
    mod.set_axon_ntff_profile_hook = lambda h: _hook.__setitem__(0, h)
    mod.get_axon_ntff_profile_hook = lambda: _hook[0]
    sys.modules["antenv.axon_hooks"] = mod
    antenv.axon_hooks = mod
    mod.set_axon_ntff_profile_hook(
        _ntff_profile_via_ctypes("/opt/axon/libaxon_pjrt.so")
    )


# ---------------------------------------------------------------------------
# device program
# ---------------------------------------------------------------------------
def _build_program():
    import concourse.bass as bass
    import concourse.mybir as mybir
    import concourse.tile as tile

    F32 = mybir.dt.float32
    F32R = mybir.dt.float32r
    AX = mybir.AxisListType
    ACTF = mybir.ActivationFunctionType

    nc = bass.Bass()

    xq_in = nc.declare_dram_parameter("xq", [C, T], F32R, isOutput=False)
    xce_in = nc.declare_dram_parameter("xce", [4, C, NKE], F32R, isOutput=False)
    xcd_in = nc.declare_dram_parameter("xcd", [4, C, NKE], F32R, isOutput=False)
    wq_in = nc.declare_dram_parameter("wq", [C, C], F32R, isOutput=False)
    wksr_in = nc.declare_dram_parameter("wksr", [4, C, C], F32R, isOutput=False)
    ck_in = nc.declare_dram_parameter("ck", [128, MB], F32, isOutput=False)
    pv_in = nc.declare_dram_parameter("pv", [128, MB], F32, isOutput=False)
    bb_in = nc.declare_dram_parameter("bb", [128, MB], F32, isOutput=False)
    id_in = nc.declare_dram_parameter("ident", [128, 128], F32R, isOutput=False)
    ones_in = nc.declare_dram_parameter("ones", [1, 128], F32R, isOutput=False)
    out_ext = nc.declare_dram_parameter("out", [C, T], F32, isOutput=True)

    sbounce = nc.dram_tensor("sbounce", [128, T // 128], F32)

    TT = T // 128            # 16 token tiles of 128
    TT4 = T // 512           # 4 token chunks of 512

    with tile.TileContext(nc) as tc:
        with tc.tile_pool(name="wts", bufs=1) as wts, \
             tc.tile_pool(name="xdat", bufs=1) as xdat, \
             tc.tile_pool(name="xqs", bufs=2) as xqs, \
             tc.tile_pool(name="work", bufs=1) as work, \
             tc.tile_pool(name="rpool", bufs=6) as rpool, \
             tc.tile_pool(name="opool", bufs=1) as opool, \
             tc.tile_pool(name="psA", bufs=3, space="PSUM") as psA, \
             tc.tile_pool(name="psD", bufs=2, space="PSUM") as psD:

            # ---- q-path inputs first so its DMAs run ahead ----
            wq_t = []
            for kc in range(KC):
                t_ = wts.tile([128, C], F32R, tag=f"wq{kc}")
                nc.sync.dma_start(out=t_[:], in_=wq_in[kc * 128:(kc + 1) * 128, :])
                wq_t.append(t_)
            ident = wts.tile([128, 128], F32R, tag="ident")
            nc.sync.dma_start(out=ident[:], in_=id_in[:])
            ones = wts.tile([1, 128], F32R, tag="ones")
            nc.sync.dma_start(out=ones[:], in_=ones_in[:])
            ck_t = wts.tile([128, MB], F32, tag="ck")
            nc.sync.dma_start(out=ck_t[:], in_=ck_in[:])
            pv_t = wts.tile([128, MB], F32, tag="pv")
            nc.sync.dma_start(out=pv_t[:], in_=pv_in[:])
            bb_t = wts.tile([128, MB], F32, tag="bb")
            nc.sync.dma_start(out=bb_t[:], in_=bb_in[:])

            # ---- persistent activations ----
            q_sb = [work.tile([128, T], F32R, tag=f"q{m}", name=f"q{m}") for m in range(MB)]
            # k2 = [k_even | k_odd]; kd = k_odd - k_even
            k2_sb = [work.tile([128, NK], F32R, tag=f"k2{m}", name=f"k2{m}") for m in range(MB)]
            kd_sb = [work.tile([128, NKE], F32R, tag=f"kd{m}", name=f"kd{m}") for m in range(MB)]
            s_acc = work.tile([128, TT * NH], F32, tag="sacc")
            s_cols = work.tile([128, TT], F32, tag="scols")
            sflat = work.tile([1, T], F32R, tag="sflat")

            # ---- P1: q projection (streamed over 512-token chunks) ----
            with nc.named_scope("q_proj"):
                for t4 in range(TT4):
                    xq_t = []
                    for kc in range(KC):
                        xt = xqs.tile([128, 512], F32R, tag=f"xq{kc}")
                        nc.sync.dma_start(
                            out=xt[:],
                            in_=xq_in[kc * 128:(kc + 1) * 128,
                                      t4 * 512:(t4 + 1) * 512])
                        xq_t.append(xt)
                    for m in range(MB):
                        pq = psA.tile([128, 1024], F32, tag="sbank")
                        for kc in range(KC):
                            nc.tensor.matmul(
                                pq[:, 0:512],
                                wq_t[kc][:, m * 128:(m + 1) * 128],
                                xq_t[kc][:],
                                start=(kc == 0), stop=(kc == KC - 1))
                        if m == 0:
                            nc.scalar.copy(
                                q_sb[m][:, t4 * 512:(t4 + 1) * 512], pq[:, 0:512])
                        else:
                            nc.vector.tensor_copy(
                                q_sb[m][:, t4 * 512:(t4 + 1) * 512], pq[:, 0:512])

            # ---- conv inputs (needed from P2 on; DMAs issued after q's) ----
            xce_t, xcd_t, wksr_t = {}, {}, {}
            for e in range(4):
                for kc in range(KC):
                    t_ = xdat.tile([128, C], F32R, tag=f"wksr{e}_{kc}",
                                   name=f"wksr{e}_{kc}")
                    nc.sync.dma_start(
                        out=t_[:], in_=wksr_in[e, kc * 128:(kc + 1) * 128, :])
                    wksr_t[(e, kc)] = t_
                    a = xdat.tile([128, NKE], F32R, tag=f"xce{e}_{kc}",
                                  name=f"xce{e}_{kc}")
                    nc.sync.dma_start(
                        out=a[:], in_=xce_in[e, kc * 128:(kc + 1) * 128, :])
                    xce_t[(e, kc)] = a
                    d = xdat.tile([128, NKE], F32R, tag=f"xcd{e}_{kc}",
                                  name=f"xcd{e}_{kc}")
                    nc.sync.dma_start(
                        out=d[:], in_=xcd_in[e, kc * 128:(kc + 1) * 128, :])
                    xcd_t[(e, kc)] = d

            # ---- P2: k_even / k_diff (2 blocks at a time, (e,kc) outer
            # so matmuls start as soon as the first conv chunks arrive) ----
            with nc.named_scope("k_proj"):
                for mh in range(2):
                    pks = [psA.tile([128, 1024], F32, tag="sbank",
                                    name=f"pk{mh}_{i}") for i in range(2)]
                    first = True
                    for e in range(4):
                        for kc in range(KC):
                            for i in range(2):
                                m = mh * 2 + i
                                nc.tensor.matmul(
                                    pks[i][:, 0:512],
                                    wksr_t[(e, kc)][:, m * 128:(m + 1) * 128],
                                    xce_t[(e, kc)][:],
                                    start=first, stop=(e == 3 and kc == KC - 1))
                                nc.tensor.matmul(
                                    pks[i][:, 512:1024],
                                    wksr_t[(e, kc)][:, m * 128:(m + 1) * 128],
                                    xcd_t[(e, kc)][:],
                                    start=first, stop=(e == 3 and kc == KC - 1))
                            first = False
                    for i in range(2):
                        m = mh * 2 + i
                        nc.scalar.activation(
                            k2_sb[m][:, 0:512], pks[i][:, 0:512], ACTF.Identity,
                            bias=ck_t[:, m:m + 1], scale=1.0)
                        nc.scalar.copy(kd_sb[m][:], pks[i][:, 512:1024])
                        nc.vector.tensor_add(
                            k2_sb[m][:, 512:1024], k2_sb[m][:, 0:512],
                            kd_sb[m][:])

            # ---- P3: scores + pair-max cascade + reduce ----
            # S slots (a-scores, later a+relu(D)) live psA; D slots live psB.
            def outer_quarter(qq):
                # tokens [qq*512, (qq+1)*512) -> flatten s, broadcast, scale
                with nc.named_scope("outer"):
                    sl = slice(qq * 4, (qq + 1) * 4)
                    tok = slice(qq * 512, (qq + 1) * 512)
                    nc.sync.dma_start(out=sbounce[:, sl], in_=s_cols[:, sl])
                    nc.gpsimd.dma_start(
                        out=sflat[0:1, tok],
                        in_=sbounce[:, sl].rearrange("p t -> () t p"))
                    pbc = psD.tile([128, 512], F32, tag="dbank",
                                   name=f"pbc{qq}")
                    nc.tensor.matmul(pbc[:], ones[:], sflat[0:1, tok],
                                     start=True, stop=True)
                    for m in range(MB):
                        osb = opool.tile([128, 512], F32, tag="osb",
                                         name=f"osb{qq}_{m}")
                        if (qq + m) % 2 == 0:
                            nc.scalar.activation(
                                osb[:], pbc[:], ACTF.Identity,
                                bias=bb_t[:, m:m + 1], scale=pv_t[:, m:m + 1])
                        else:
                            nc.vector.tensor_scalar(
                                out=osb[:], in0=pbc[:],
                                scalar1=pv_t[:, m:m + 1],
                                scalar2=bb_t[:, m:m + 1],
                                op0=mybir.AluOpType.mult,
                                op1=mybir.AluOpType.add)
                        nc.sync.dma_start(
                            out=out_ext[m * 128:(m + 1) * 128, tok],
                            in_=osb[:])

            # Software-pipelined emission, all-trick. Per step g:
            #   relus of g-1 (ScalarE, early so PE's D-mms of g+1 are safe),
            #   S/D matmuls of g (PE; D slots are per-head 1-bank tiles),
            #   identity-accumulates of g-2 (PE; relus finished a step ago),
            #   reduce of g-2 (VectorE).
            # S-slots: 3x2 banks; D-slots: 2x1 bank -> 8 PSUM banks.
            NG = TT * MB
            state = {}

            def emit_relus(g):
                pS, pDa, pDb, rr = state[g]
                ra = rpool.tile([128, 512], F32R, tag="r", name=f"ra{g}")
                rb = rpool.tile([128, 512], F32R, tag="r", name=f"rb{g}")
                nc.scalar.activation(ra[:], pDa[:], ACTF.Relu)
                nc.scalar.activation(rb[:], pDb[:], ACTF.Relu)
                state[g] = (pS, pDa, pDb, (ra, rb))

            def emit_front(g):
                tt, m = divmod(g, MB)
                qs = q_sb[m]
                tsl = slice(tt * 128, (tt + 1) * 128)
                pS = psA.tile([128, 1024], F32, tag="sbank", name=f"pS{g}")
                pDa = psD.tile([128, 512], F32, tag="dbank", name=f"pDa{g}")
                pDb = psD.tile([128, 512], F32, tag="dbank", name=f"pDb{g}")
                nc.tensor.matmul(pS[:, 0:512], qs[0:64, tsl],
                                 k2_sb[m][0:64, 0:512], start=True,
                                 stop=True, tile_position=(0, 0))
                nc.tensor.matmul(pS[:, 512:1024], qs[64:128, tsl],
                                 k2_sb[m][64:128, 0:512], start=True,
                                 stop=True, tile_position=(64, 0))
                nc.tensor.matmul(pDa[:], qs[0:64, tsl],
                                 kd_sb[m][0:64, :], start=True,
                                 stop=True, tile_position=(0, 0))
                nc.tensor.matmul(pDb[:], qs[64:128, tsl],
                                 kd_sb[m][64:128, :], start=True,
                                 stop=True, tile_position=(64, 0))
                state[g] = (pS, pDa, pDb, None)

            def emit_iadd(g):
                # one identity-accumulate adds both heads' relu corrections
                # (rhs spans 1024 columns -> two PSUM banks)
                pS, pDa, pDb, (ra, rb) = state[g]
                nc.tensor.matmul(pS[:, 0:512], ident[:], ra[:],
                                 start=False, stop=True)
                nc.tensor.matmul(pS[:, 512:1024], ident[:], rb[:],
                                 start=False, stop=True)

            def emit_back(g):
                tt, m = divmod(g, MB)
                pS = state.pop(g)[0]
                cols = slice(tt * NH + 2 * m, tt * NH + 2 * m + 2)
                nc.vector.reduce_max(
                    s_acc[:, cols],
                    pS[:].rearrange("p (a b) -> p a b", a=2), axis=AX.X)
                if m == MB - 1:
                    nc.vector.reduce_sum(
                        s_cols[:, tt:tt + 1],
                        s_acc[:, tt * NH:(tt + 1) * NH], axis=AX.X)
                    if tt in (3, 7, 11, TT - 1):
                        outer_quarter(tt // 4)

            with nc.named_scope("scores"):
                for g in range(NG + 2):
                    if g < NG:
                        emit_front(g)
                        emit_relus(g)
                    if g >= 2:
                        emit_iadd(g - 2)
                        emit_back(g - 2)

    _split_multi_waits(nc)
    return nc


# ---------------------------------------------------------------------------
# host side
# ---------------------------------------------------------------------------
def _prep_host(x, Wq, Wk, Wsr, bsr, bn_gamma, bn_beta, bn_mean, bn_var,
               Wproj, bproj):
    f8 = np.float64
    scale = HD ** -0.5
    g = bn_gamma.astype(f8) / np.sqrt(bn_var.astype(f8) + EPS)
    A = Wk.astype(f8) * g[None, :]
    ck = A @ bsr.astype(f8) + Wk.astype(f8) @ (
        bn_beta.astype(f8) - bn_mean.astype(f8) * g)
    wksr = np.stack([
        (A @ Wsr[:, :, e // 2, e % 2].astype(f8)).T for e in range(4)
    ]).astype(np.float32)                              # (4, C_in, C_out)
    wqT = (Wq.astype(f8) * scale).T.astype(np.float32)  # (C_in, C_out)

    x4 = x.reshape(B, C, HW_, HW_)
    xce = np.empty((B, 4, C, NKE), np.float32)
    xcd = np.empty((B, 4, C, NKE), np.float32)
    for e in range(4):
        di, dj = e // 2, e % 2
        even = x4[:, :, di::2, dj::4].reshape(B, C, NKE)
        odd = x4[:, :, di::2, dj + 2::4].reshape(B, C, NKE)
        xce[:, e] = even
        xcd[:, e] = odd - even

    v = x.astype(f8).mean(axis=2)                       # (B, C)
    pv = (Wproj.astype(f8) @ v.T).T.astype(np.float32)  # (B, C)

    ck_t = ck.astype(np.float32).reshape(MB, 128).T.copy()    # (128, MB)
    bb_t = bproj.astype(np.float32).reshape(MB, 128).T.copy()
    pv_t = [pv[b].reshape(MB, 128).T.copy() for b in range(B)]
    return wqT, wksr, ck_t, bb_t, pv_t, xce, xcd


def kernel(x, y, Wq, Wk, Wsr, bsr, bn_gamma, bn_beta, bn_mean, bn_var,
           Wproj, bproj, H, W):
    x = np.asarray(x, np.float32)
    wqT, wksr, ck_t, bb_t, pv_t, xce, xcd = _prep_host(
        x, np.asarray(Wq, np.float32), np.asarray(Wk, np.float32),
        np.asarray(Wsr, np.float32), np.asarray(bsr, np.float32),
        np.asarray(bn_gamma, np.float32), np.asarray(bn_beta, np.float32),
        np.asarray(bn_mean, np.float32), np.asarray(bn_var, np.float32),
        np.asarray(Wproj, np.float32), np.asarray(bproj, np.float32))

    _install_fixes()
    _install_ntff_hook()
    from concourse.bass_utils import run_bass_kernel_spmd

    if "nc" not in _cache:
        _cache["nc"] = _build_program()
    nc = _cache["nc"]

    ident = np.eye(128, dtype=np.float32)
    ones = np.ones((1, 128), np.float32)
    in_maps = []
    for core in range(NCORES):
        b, half = core // 2, core % 2
        in_maps.append({
            "xq": np.ascontiguousarray(x[b][:, half * T:(half + 1) * T]),
            "xce": xce[b], "xcd": xcd[b],
            "wq": wqT, "wksr": wksr,
            "ck": ck_t, "pv": pv_t[b], "bb": bb_t,
            "ident": ident, "ones": ones,
        })

    trace = os.environ.get("BASS_KERNEL_TRACE", "0") == "1"
    res = run_bass_kernel_spmd(nc, in_maps, list(range(NCORES)), trace=trace)
    if trace:
        print(f"HW exec time: {res.exec_time_ns} ns")
        _cache["last_exec_time_ns"] = res.exec_time_ns
        _cache["last_trace"] = res.instructions_and_trace

    out = np.empty((B, C, N), np.float32)
    for core in range(NCORES):
        b, half = core // 2, core % 2
        out[b][:, half * T:(half + 1) * T] = res.results[core]["out"]
    return out


# revision 23
# speedup vs baseline: 1.3634x; 1.3634x over previous
"""Trainium2 Bass kernel for nn_Attention_Joint_MaxPool.

Math (see reference):
  q = (Wq*scale) @ x                        (B, C, N), heads on rows
  xsr = conv2x2s2(x) ; k = Wk @ BN(xsr)     (B, C, Nk=1024)
  attn = max over keys of q_h . k_h         (B, NH, N)
  s = sum over heads of attn                (B, N)
  out[b,c,n] = (Wproj @ mean_n x)[c] * s[b,n] + bproj[c]

Weight folding done on host:
  g = gamma/sqrt(var+eps); A = Wk * g[None,:]
  k = sum_e (A @ Wsr[:,:,e]) @ x_sub[e] + ck,  ck = A@bsr + Wk@(beta-mean*g)
  pv[b] = Wproj @ mean_n x[b]   (rank-1 output structure)

Max over keys via pair cascade: for key pairs (even, odd):
  max(a,b) = a + relu(b-a); a = q.k_even, (b-a) = q.(k_odd-k_even)
  k_even and k_diff both come from matmuls of host-prearranged x columns,
  relu on ScalarE, the add is a PE identity-matmul accumulate into PSUM,
  then one VectorE reduce_max per head over the 512 pair-maxes.

Sharding: 8 cores; core i -> batch i//2, token half i%2 (2048 tokens).
Each core is fully independent (no collectives).
"""

import os
import sys
import types
import numpy as np

# ---------------------------------------------------------------------------
# problem constants (hardcoded; kernel.py must be self-contained)
# ---------------------------------------------------------------------------
B, C, N = 4, 512, 4096
NH, HD = 8, 64
SR = 2
EPS = 1e-5
HW_ = 64                      # H = W = 64
T = N // 2                    # tokens per core
NK = 1024                     # conv output positions (keys)
NKE = NK // 2                 # even keys (pairs)
MB = C // 128                 # 4 channel blocks
KC = C // 128                 # 4 contraction chunks
NCORES = 8

_cache = {}


# ---------------------------------------------------------------------------
# workarounds for this container's toolchain
# ---------------------------------------------------------------------------
def _install_fixes():
    import concourse.tile as tile
    import concourse.mybir as mybir
    from concourse.vector_clock import ScopedClock

    if getattr(tile.TileContext, "_drain_patched", False):
        return

    def _patched_drain_and_barrier(self, tick_clock, wait_clock):
        nc = self.nc
        probe = nc.sync.nop(nofuse=True, hint="drain_wait_carrier")
        wait_clock.add_sem_waits(
            probe.ins, ScopedClock({None: tick_clock.global_clock})
        )
        waits = list(probe.ins.sync_info.on_wait) if probe.ins.sync_info else []
        if len(waits) > 1:
            probe.ins.sync_info = mybir.SyncInfo(on_wait=waits[:1], on_update=[])
            for w in waits[1:]:
                extra = nc.sync.nop(nofuse=True, hint="drain_wait_carrier")
                extra.ins.sync_info = mybir.SyncInfo(on_wait=[w], on_update=[])
        nc.sync.drain()
        nc.all_engine_barrier()
        assert self.sems is not None
        popped = nc._tile_sem_poison_stack.pop()
        assert popped is self._sem_poison
        nc.clear_and_free_semaphores(list(self.sems.allocated().values()))
        nc.all_engine_barrier()

    tile.TileContext._drain_and_barrier = _patched_drain_and_barrier
    tile.TileContext._drain_patched = True


def _split_multi_waits(nc):
    """This walrus build allows only one sync-wait per instruction; hoist
    extra waits onto same-engine nops inserted just before the instruction."""
    import concourse.mybir as mybir

    ctr = 0
    for f in nc.m.functions:
        for bb in f.blocks:
            changed = False
            out = []
            for inst in bb.instructions:
                si = inst.sync_info
                tname = type(inst).__name__
                if (si is not None and si.on_wait and len(si.on_wait) > 1
                        and "Collective" not in tname):
                    waits = list(si.on_wait)
                    for w in waits[:-1]:
                        ctr += 1
                        nop = mybir.InstNoOp(
                            name=f"I-ws-{ctr}",
                            engine=inst.engine,
                            sync_info=mybir.SyncInfo(on_wait=[w], on_update=[]),
                        )
                        nc.register_instruction(nop, overwrite=True)
                        out.append(nop)
                    inst.sync_info = mybir.SyncInfo(
                        on_wait=waits[-1:], on_update=list(si.on_update)
                    )
                    changed = True
                out.append(inst)
            if changed:
                bb.instructions = out


def _install_ntff_hook():
    """Provide antenv.axon_hooks (missing in this image) so trace=True works."""
    try:
        from antenv import axon_hooks  # noqa: F401
        return
    except ImportError:
        pass
    try:
        import antenv
        from trn_agent_boot.trn_boot import _ntff_profile_via_ctypes
    except ImportError:
        return
    mod = types.ModuleType("antenv.axon_hooks")
    _hook = [None]
    mod.set_axon_ntff_profile_hook = lambda h: _hook.__setitem__(0, h)
    mod.get_axon_ntff_profile_hook = lambda: _hook[0]
    sys.modules["antenv.axon_hooks"] = mod
    antenv.axon_hooks = mod
    mod.set_axon_ntff_profile_hook(
        _ntff_profile_via_ctypes("/opt/axon/libaxon_pjrt.so")
    )


# ---------------------------------------------------------------------------
# device program
# ---------------------------------------------------------------------------
def _build_program():
    import concourse.bass as bass
    import concourse.mybir as mybir
    import concourse.tile as tile

    F32 = mybir.dt.float32
    F32R = mybir.dt.float32r
    AX = mybir.AxisListType
    ACTF = mybir.ActivationFunctionType

    nc = bass.Bass()

    xq_in = nc.declare_dram_parameter("xq", [C, T], F32R, isOutput=False)
    xce_in = nc.declare_dram_parameter("xce", [4, C, NKE], F32R, isOutput=False)
    xcd_in = nc.declare_dram_parameter("xcd", [4, C, NKE], F32R, isOutput=False)
    wq_in = nc.declare_dram_parameter("wq", [C, C], F32R, isOutput=False)
    wksr_in = nc.declare_dram_parameter("wksr", [4, C, C], F32R, isOutput=False)
    ck_in = nc.declare_dram_parameter("ck", [128, MB], F32, isOutput=False)
    pv_in = nc.declare_dram_parameter("pv", [128, MB], F32, isOutput=False)
    bb_in = nc.declare_dram_parameter("bb", [128, MB], F32, isOutput=False)
    id_in = nc.declare_dram_parameter("ident", [128, 128], F32R, isOutput=False)
    ones_in = nc.declare_dram_parameter("ones", [1, 128], F32R, isOutput=False)
    out_ext = nc.declare_dram_parameter("out", [C, T], F32, isOutput=True)

    sbounce = nc.dram_tensor("sbounce", [128, T // 128], F32)

    TT = T // 128            # 16 token tiles of 128
    TT4 = T // 512           # 4 token chunks of 512

    with tile.TileContext(nc) as tc:
        with tc.tile_pool(name="wts", bufs=1) as wts, \
             tc.tile_pool(name="xdat", bufs=1) as xdat, \
             tc.tile_pool(name="xqs", bufs=2) as xqs, \
             tc.tile_pool(name="work", bufs=1) as work, \
             tc.tile_pool(name="rpool", bufs=6) as rpool, \
             tc.tile_pool(name="opool", bufs=1) as opool, \
             tc.tile_pool(name="psA", bufs=3, space="PSUM") as psA, \
             tc.tile_pool(name="psD", bufs=2, space="PSUM") as psD:

            # ---- q-path inputs first so its DMAs run ahead ----
            wq_t = []
            for kc in range(KC):
                t_ = wts.tile([128, C], F32R, tag=f"wq{kc}")
                nc.sync.dma_start(out=t_[:], in_=wq_in[kc * 128:(kc + 1) * 128, :])
                wq_t.append(t_)
            ident = wts.tile([128, 128], F32R, tag="ident")
            nc.sync.dma_start(out=ident[:], in_=id_in[:])
            ones = wts.tile([1, 128], F32R, tag="ones")
            nc.sync.dma_start(out=ones[:], in_=ones_in[:])
            ck_t = wts.tile([128, MB], F32, tag="ck")
            nc.sync.dma_start(out=ck_t[:], in_=ck_in[:])
            pv_t = wts.tile([128, MB], F32, tag="pv")
            nc.sync.dma_start(out=pv_t[:], in_=pv_in[:])
            bb_t = wts.tile([128, MB], F32, tag="bb")
            nc.sync.dma_start(out=bb_t[:], in_=bb_in[:])

            # ---- persistent activations ----
            q_sb = [work.tile([128, T], F32R, tag=f"q{m}", name=f"q{m}") for m in range(MB)]
            # k2 = [k_even | k_odd]; kd = k_odd - k_even
            k2_sb = [work.tile([128, NK], F32R, tag=f"k2{m}", name=f"k2{m}") for m in range(MB)]
            kd_sb = [work.tile([128, NKE], F32R, tag=f"kd{m}", name=f"kd{m}") for m in range(MB)]
            s_acc = work.tile([128, TT * NH], F32, tag="sacc")
            s_cols = work.tile([128, TT], F32, tag="scols")
            sflat = work.tile([1, T], F32R, tag="sflat")

            # ---- P1: q projection (streamed over 512-token chunks) ----
            with nc.named_scope("q_proj"):
                for t4 in range(TT4):
                    xq_t = []
                    for kc in range(KC):
                        xt = xqs.tile([128, 512], F32R, tag=f"xq{kc}")
                        nc.sync.dma_start(
                            out=xt[:],
                            in_=xq_in[kc * 128:(kc + 1) * 128,
                                      t4 * 512:(t4 + 1) * 512])
                        xq_t.append(xt)
                    for m in range(MB):
                        pq = psA.tile([128, 1024], F32, tag="sbank")
                        for kc in range(KC):
                            nc.tensor.matmul(
                                pq[:, 0:512],
                                wq_t[kc][:, m * 128:(m + 1) * 128],
                                xq_t[kc][:],
                                start=(kc == 0), stop=(kc == KC - 1))
                        if m % 2 == 0:
                            nc.scalar.copy(
                                q_sb[m][:, t4 * 512:(t4 + 1) * 512], pq[:, 0:512])
                        else:
                            nc.vector.tensor_copy(
                                q_sb[m][:, t4 * 512:(t4 + 1) * 512], pq[:, 0:512])

            # ---- conv inputs (needed from P2 on; DMAs issued after q's) ----
            xce_t, xcd_t, wksr_t = {}, {}, {}
            for e in range(4):
                for kc in range(KC):
                    t_ = xdat.tile([128, C], F32R, tag=f"wksr{e}_{kc}",
                                   name=f"wksr{e}_{kc}")
                    nc.sync.dma_start(
                        out=t_[:], in_=wksr_in[e, kc * 128:(kc + 1) * 128, :])
                    wksr_t[(e, kc)] = t_
                    a = xdat.tile([128, NKE], F32R, tag=f"xce{e}_{kc}",
                                  name=f"xce{e}_{kc}")
                    nc.sync.dma_start(
                        out=a[:], in_=xce_in[e, kc * 128:(kc + 1) * 128, :])
                    xce_t[(e, kc)] = a
                    d = xdat.tile([128, NKE], F32R, tag=f"xcd{e}_{kc}",
                                  name=f"xcd{e}_{kc}")
                    nc.sync.dma_start(
                        out=d[:], in_=xcd_in[e, kc * 128:(kc + 1) * 128, :])
                    xcd_t[(e, kc)] = d

            # ---- P2: k_even / k_diff (2 blocks at a time, (e,kc) outer
            # so matmuls start as soon as the first conv chunks arrive) ----
            with nc.named_scope("k_proj"):
                for mh in range(2):
                    pks = [psA.tile([128, 1024], F32, tag="sbank",
                                    name=f"pk{mh}_{i}") for i in range(2)]
                    first = True
                    for e in range(4):
                        for kc in range(KC):
                            for i in range(2):
                                m = mh * 2 + i
                                nc.tensor.matmul(
                                    pks[i][:, 0:512],
                                    wksr_t[(e, kc)][:, m * 128:(m + 1) * 128],
                                    xce_t[(e, kc)][:],
                                    start=first, stop=(e == 3 and kc == KC - 1))
                                nc.tensor.matmul(
                                    pks[i][:, 512:1024],
                                    wksr_t[(e, kc)][:, m * 128:(m + 1) * 128],
                                    xcd_t[(e, kc)][:],
                                    start=first, stop=(e == 3 and kc == KC - 1))
                            first = False
                    for i in range(2):
                        m = mh * 2 + i
                        nc.scalar.activation(
                            k2_sb[m][:, 0:512], pks[i][:, 0:512], ACTF.Identity,
                            bias=ck_t[:, m:m + 1], scale=1.0)
                        nc.scalar.copy(kd_sb[m][:], pks[i][:, 512:1024])
                        nc.vector.tensor_add(
                            k2_sb[m][:, 512:1024], k2_sb[m][:, 0:512],
                            kd_sb[m][:])

            # ---- P3: scores + pair-max cascade + reduce ----
            # S slots (a-scores, later a+relu(D)) live psA; D slots live psB.
            def outer_half(hh):
                with nc.named_scope("outer"):
                    sl = slice(hh * (TT // 2), (hh + 1) * (TT // 2))
                    nc.sync.dma_start(out=sbounce[:, sl], in_=s_cols[:, sl])
                    nc.gpsimd.dma_start(
                        out=sflat[0:1, hh * (T // 2):(hh + 1) * (T // 2)],
                        in_=sbounce[:, sl].rearrange("p t -> () t p"))
                    pbc = psA.tile([128, 1024], F32, tag="sbank",
                                   name=f"pbc{hh}")
                    for t2 in range(2):
                        nc.tensor.matmul(
                            pbc[:, t2 * 512:(t2 + 1) * 512], ones[:],
                            sflat[0:1,
                                  hh * (T // 2) + t2 * 512:
                                  hh * (T // 2) + (t2 + 1) * 512],
                            start=True, stop=True)
                    for m in range(MB):
                        osb = opool.tile([128, T // 2], F32, tag="osb",
                                         name=f"osb{hh}_{m}")
                        nc.scalar.activation(
                            osb[:], pbc[:], ACTF.Identity,
                            bias=bb_t[:, m:m + 1], scale=pv_t[:, m:m + 1])
                        nc.sync.dma_start(
                            out=out_ext[m * 128:(m + 1) * 128,
                                        hh * (T // 2):(hh + 1) * (T // 2)],
                            in_=osb[:])

            # Software-pipelined emission, all-trick. Per step g:
            #   relus of g-1 (ScalarE, early so PE's D-mms of g+1 are safe),
            #   S/D matmuls of g (PE; D slots are per-head 1-bank tiles),
            #   identity-accumulates of g-2 (PE; relus finished a step ago),
            #   reduce of g-2 (VectorE).
            # S-slots: 3x2 banks; D-slots: 2x1 bank -> 8 PSUM banks.
            NG = TT * MB
            state = {}

            def emit_relus(g):
                pS, pDa, pDb, rr = state[g]
                ra = rpool.tile([128, 512], F32R, tag="r", name=f"ra{g}")
                rb = rpool.tile([128, 512], F32R, tag="r", name=f"rb{g}")
                nc.scalar.activation(ra[:], pDa[:], ACTF.Relu)
                nc.scalar.activation(rb[:], pDb[:], ACTF.Relu)
                state[g] = (pS, pDa, pDb, (ra, rb))

            def emit_front(g):
                tt, m = divmod(g, MB)
                qs = q_sb[m]
                tsl = slice(tt * 128, (tt + 1) * 128)
                pS = psA.tile([128, 1024], F32, tag="sbank", name=f"pS{g}")
                pDa = psD.tile([128, 512], F32, tag="dbank", name=f"pDa{g}")
                pDb = psD.tile([128, 512], F32, tag="dbank", name=f"pDb{g}")
                nc.tensor.matmul(pS[:, 0:512], qs[0:64, tsl],
                                 k2_sb[m][0:64, 0:512], start=True,
                                 stop=True, tile_position=(0, 0))
                nc.tensor.matmul(pS[:, 512:1024], qs[64:128, tsl],
                                 k2_sb[m][64:128, 0:512], start=True,
                                 stop=True, tile_position=(64, 0))
                nc.tensor.matmul(pDa[:], qs[0:64, tsl],
                                 kd_sb[m][0:64, :], start=True,
                                 stop=True, tile_position=(0, 0))
                nc.tensor.matmul(pDb[:], qs[64:128, tsl],
                                 kd_sb[m][64:128, :], start=True,
                                 stop=True, tile_position=(64, 0))
                state[g] = (pS, pDa, pDb, None)

            def emit_iadd(g):
                # one identity-accumulate adds both heads' relu corrections
                # (rhs spans 1024 columns -> two PSUM banks)
                pS, pDa, pDb, (ra, rb) = state[g]
                nc.tensor.matmul(pS[:, 0:512], ident[:], ra[:],
                                 start=False, stop=True)
                nc.tensor.matmul(pS[:, 512:1024], ident[:], rb[:],
                                 start=False, stop=True)

            def emit_back(g):
                tt, m = divmod(g, MB)
                pS = state.pop(g)[0]
                cols = slice(tt * NH + 2 * m, tt * NH + 2 * m + 2)
                nc.vector.reduce_max(
                    s_acc[:, cols],
                    pS[:].rearrange("p (a b) -> p a b", a=2), axis=AX.X)
                if m == MB - 1:
                    nc.vector.reduce_sum(
                        s_cols[:, tt:tt + 1],
                        s_acc[:, tt * NH:(tt + 1) * NH], axis=AX.X)
                    if tt == 9:
                        outer_half(0)
                    elif tt == TT - 1:
                        outer_half(1)

            with nc.named_scope("scores"):
                for g in range(NG + 2):
                    if g < NG:
                        emit_front(g)
                        emit_relus(g)
                    if g >= 2:
                        emit_iadd(g - 2)
                        emit_back(g - 2)

    _split_multi_waits(nc)
    return nc


# ---------------------------------------------------------------------------
# host side
# ---------------------------------------------------------------------------
def _prep_host(x, Wq, Wk, Wsr, bsr, bn_gamma, bn_beta, bn_mean, bn_var,
               Wproj, bproj):
    f8 = np.float64
    scale = HD ** -0.5
    g = bn_gamma.astype(f8) / np.sqrt(bn_var.astype(f8) + EPS)
    A = Wk.astype(f8) * g[None, :]
    ck = A @ bsr.astype(f8) + Wk.astype(f8) @ (
        bn_beta.astype(f8) - bn_mean.astype(f8) * g)
    wksr = np.stack([
        (A @ Wsr[:, :, e // 2, e % 2].astype(f8)).T for e in range(4)
    ]).astype(np.float32)                              # (4, C_in, C_out)
    wqT = (Wq.astype(f8) * scale).T.astype(np.float32)  # (C_in, C_out)

    x4 = x.reshape(B, C, HW_, HW_)
    xce = np.empty((B, 4, C, NKE), np.float32)
    xcd = np.empty((B, 4, C, NKE), np.float32)
    for e in range(4):
        di, dj = e // 2, e % 2
        even = x4[:, :, di::2, dj::4].reshape(B, C, NKE)
        odd = x4[:, :, di::2, dj + 2::4].reshape(B, C, NKE)
        xce[:, e] = even
        xcd[:, e] = odd - even

    v = x.astype(f8).mean(axis=2)                       # (B, C)
    pv = (Wproj.astype(f8) @ v.T).T.astype(np.float32)  # (B, C)

    ck_t = ck.astype(np.float32).reshape(MB, 128).T.copy()    # (128, MB)
    bb_t = bproj.astype(np.float32).reshape(MB, 128).T.copy()
    pv_t = [pv[b].reshape(MB, 128).T.copy() for b in range(B)]
    return wqT, wksr, ck_t, bb_t, pv_t, xce, xcd


def kernel(x, y, Wq, Wk, Wsr, bsr, bn_gamma, bn_beta, bn_mean, bn_var,
           Wproj, bproj, H, W):
    x = np.asarray(x, np.float32)
    wqT, wksr, ck_t, bb_t, pv_t, xce, xcd = _prep_host(
        x, np.asarray(Wq, np.float32), np.asarray(Wk, np.float32),
        np.asarray(Wsr, np.float32), np.asarray(bsr, np.float32),
        np.asarray(bn_gamma, np.float32), np.asarray(bn_beta, np.float32),
        np.asarray(bn_mean, np.float32), np.asarray(bn_var, np.float32),
        np.asarray(Wproj, np.float32), np.asarray(bproj, np.float32))

    _install_fixes()
    _install_ntff_hook()
    from concourse.bass_utils import run_bass_kernel_spmd

    if "nc" not in _cache:
        _cache["nc"] = _build_program()
    nc = _cache["nc"]

    ident = np.eye(128, dtype=np.float32)
    ones = np.ones((1, 128), np.float32)
    in_maps = []
    for core in range(NCORES):
        b, half = core // 2, core % 2
        in_maps.append({
            "xq": np.ascontiguousarray(x[b][:, half * T:(half + 1) * T]),
            "xce": xce[b], "xcd": xcd[b],
            "wq": wqT, "wksr": wksr,
            "ck": ck_t, "pv": pv_t[b], "bb": bb_t,
            "ident": ident, "ones": ones,
        })

    trace = os.environ.get("BASS_KERNEL_TRACE", "0") == "1"
    res = run_bass_kernel_spmd(nc, in_maps, list(range(NCORES)), trace=trace)
    if trace:
        print(f"HW exec time: {res.exec_time_ns} ns")
        _cache["last_exec_time_ns"] = res.exec_time_ns
        _cache["last_trace"] = res.instructions_and_trace

    out = np.empty((B, C, N), np.float32)
    for core in range(NCORES):
        b, half = core // 2, core % 2
        out[b][:, half * T:(half + 1) * T] = res.results[core]["out"]
    return out


# revision 24
# speedup vs baseline: 1.4568x; 1.0685x over previous
"""Trainium2 Bass kernel for nn_Attention_Joint_MaxPool.

Math (see reference):
  q = (Wq*scale) @ x                        (B, C, N), heads on rows
  xsr = conv2x2s2(x) ; k = Wk @ BN(xsr)     (B, C, Nk=1024)
  attn = max over keys of q_h . k_h         (B, NH, N)
  s = sum over heads of attn                (B, N)
  out[b,c,n] = (Wproj @ mean_n x)[c] * s[b,n] + bproj[c]

Weight folding done on host:
  g = gamma/sqrt(var+eps); A = Wk * g[None,:]
  k = sum_e (A @ Wsr[:,:,e]) @ x_sub[e] + ck,  ck = A@bsr + Wk@(beta-mean*g)
  pv[b] = Wproj @ mean_n x[b]   (rank-1 output structure)

Max over keys via pair cascade: for key pairs (even, odd):
  max(a,b) = a + relu(b-a); a = q.k_even, (b-a) = q.(k_odd-k_even)
  k_even and k_diff both come from matmuls of host-prearranged x columns,
  relu on ScalarE, the add is a PE identity-matmul accumulate into PSUM,
  then one VectorE reduce_max per head over the 512 pair-maxes.

Sharding: 8 cores; core i -> batch i//2, token half i%2 (2048 tokens).
Each core is fully independent (no collectives).
"""

import os
import sys
import types
import numpy as np

# ---------------------------------------------------------------------------
# problem constants (hardcoded; kernel.py must be self-contained)
# ---------------------------------------------------------------------------
B, C, N = 4, 512, 4096
NH, HD = 8, 64
SR = 2
EPS = 1e-5
HW_ = 64                      # H = W = 64
T = N // 2                    # tokens per core
NK = 1024                     # conv output positions (keys)
NKE = NK // 2                 # even keys (pairs)
MB = C // 128                 # 4 channel blocks
KC = C // 128                 # 4 contraction chunks
NCORES = 8

_cache = {}


# ---------------------------------------------------------------------------
# workarounds for this container's toolchain
# ---------------------------------------------------------------------------
def _install_fixes():
    import concourse.tile as tile
    import concourse.mybir as mybir
    from concourse.vector_clock import ScopedClock

    if getattr(tile.TileContext, "_drain_patched", False):
        return

    def _patched_drain_and_barrier(self, tick_clock, wait_clock):
        nc = self.nc
        probe = nc.sync.nop(nofuse=True, hint="drain_wait_carrier")
        wait_clock.add_sem_waits(
            probe.ins, ScopedClock({None: tick_clock.global_clock})
        )
        waits = list(probe.ins.sync_info.on_wait) if probe.ins.sync_info else []
        if len(waits) > 1:
            probe.ins.sync_info = mybir.SyncInfo(on_wait=waits[:1], on_update=[])
            for w in waits[1:]:
                extra = nc.sync.nop(nofuse=True, hint="drain_wait_carrier")
                extra.ins.sync_info = mybir.SyncInfo(on_wait=[w], on_update=[])
        nc.sync.drain()
        nc.all_engine_barrier()
        assert self.sems is not None
        popped = nc._tile_sem_poison_stack.pop()
        assert popped is self._sem_poison
        nc.clear_and_free_semaphores(list(self.sems.allocated().values()))
        nc.all_engine_barrier()

    tile.TileContext._drain_and_barrier = _patched_drain_and_barrier
    tile.TileContext._drain_patched = True


def _split_multi_waits(nc):
    """This walrus build allows only one sync-wait per instruction; hoist
    extra waits onto same-engine nops inserted just before the instruction."""
    import concourse.mybir as mybir

    ctr = 0
    for f in nc.m.functions:
        for bb in f.blocks:
            changed = False
            out = []
            for inst in bb.instructions:
                si = inst.sync_info
                tname = type(inst).__name__
                if (si is not None and si.on_wait and len(si.on_wait) > 1
                        and "Collective" not in tname):
                    waits = list(si.on_wait)
                    for w in waits[:-1]:
                        ctr += 1
                        nop = mybir.InstNoOp(
                            name=f"I-ws-{ctr}",
                            engine=inst.engine,
                            sync_info=mybir.SyncInfo(on_wait=[w], on_update=[]),
                        )
                        nc.register_instruction(nop, overwrite=True)
                        out.append(nop)
                    inst.sync_info = mybir.SyncInfo(
                        on_wait=waits[-1:], on_update=list(si.on_update)
                    )
                    changed = True
                out.append(inst)
            if changed:
                bb.instructions = out


def _install_ntff_hook():
    """Provide antenv.axon_hooks (missing in this image) so trace=True works."""
    try:
        from antenv import axon_hooks  # noqa: F401
        return
    except ImportError:
        pass
    try:
        import antenv
        from trn_agent_boot.trn_boot import _ntff_profile_via_ctypes
    except ImportError:
        return
    mod = types.ModuleType("antenv.axon_hooks")
    _hook = [None]
    mod.set_axon_ntff_profile_hook = lambda h: _hook.__setitem__(0, h)
    mod.get_axon_ntff_profile_hook = lambda: _hook[0]
    sys.modules["antenv.axon_hooks"] = mod
    antenv.axon_hooks = mod
    mod.set_axon_ntff_profile_hook(
        _ntff_profile_via_ctypes("/opt/axon/libaxon_pjrt.so")
    )


# ---------------------------------------------------------------------------
# device program
# ---------------------------------------------------------------------------
def _build_program():
    import concourse.bass as bass
    import concourse.mybir as mybir
    import concourse.tile as tile

    F32 = mybir.dt.float32
    F32R = mybir.dt.float32r
    AX = mybir.AxisListType
    ACTF = mybir.ActivationFunctionType

    nc = bass.Bass()

    xq_in = nc.declare_dram_parameter("xq", [C, T], F32R, isOutput=False)
    xce_in = nc.declare_dram_parameter("xce", [4, C, NKE], F32R, isOutput=False)
    xcd_in = nc.declare_dram_parameter("xcd", [4, C, NKE], F32R, isOutput=False)
    wq_in = nc.declare_dram_parameter("wq", [C, C], F32R, isOutput=False)
    wksr_in = nc.declare_dram_parameter("wksr", [4, C, C], F32R, isOutput=False)
    ck_in = nc.declare_dram_parameter("ck", [128, MB], F32, isOutput=False)
    pv_in = nc.declare_dram_parameter("pv", [128, MB], F32, isOutput=False)
    bb_in = nc.declare_dram_parameter("bb", [128, MB], F32, isOutput=False)
    id_in = nc.declare_dram_parameter("ident", [128, 128], F32R, isOutput=False)
    ones_in = nc.declare_dram_parameter("ones", [1, 128], F32R, isOutput=False)
    out_ext = nc.declare_dram_parameter("out", [C, T], F32, isOutput=True)

    sbounce = nc.dram_tensor("sbounce", [128, T // 128], F32)

    TT = T // 128            # 16 token tiles of 128
    TT4 = T // 512           # 4 token chunks of 512

    with tile.TileContext(nc) as tc:
        with tc.tile_pool(name="wts", bufs=1) as wts, \
             tc.tile_pool(name="xdat", bufs=1) as xdat, \
             tc.tile_pool(name="xqs", bufs=2) as xqs, \
             tc.tile_pool(name="work", bufs=1) as work, \
             tc.tile_pool(name="rpool", bufs=6) as rpool, \
             tc.tile_pool(name="opool", bufs=1) as opool, \
             tc.tile_pool(name="psA", bufs=3, space="PSUM") as psA, \
             tc.tile_pool(name="psD", bufs=2, space="PSUM") as psD:

            # ---- q-path inputs first so its DMAs run ahead ----
            wq_t = []
            for kc in range(KC):
                t_ = wts.tile([128, C], F32R, tag=f"wq{kc}")
                nc.sync.dma_start(out=t_[:], in_=wq_in[kc * 128:(kc + 1) * 128, :])
                wq_t.append(t_)
            ident = wts.tile([128, 128], F32R, tag="ident")
            nc.sync.dma_start(out=ident[:], in_=id_in[:])
            ones = wts.tile([1, 128], F32R, tag="ones")
            nc.sync.dma_start(out=ones[:], in_=ones_in[:])
            ck_t = wts.tile([128, MB], F32, tag="ck")
            nc.sync.dma_start(out=ck_t[:], in_=ck_in[:])
            pv_t = wts.tile([128, MB], F32, tag="pv")
            nc.sync.dma_start(out=pv_t[:], in_=pv_in[:])
            bb_t = wts.tile([128, MB], F32, tag="bb")
            nc.sync.dma_start(out=bb_t[:], in_=bb_in[:])

            # ---- persistent activations ----
            q_sb = [work.tile([128, T], F32R, tag=f"q{m}", name=f"q{m}") for m in range(MB)]
            # k2 = [k_even | k_odd]; kd = k_odd - k_even
            k2_sb = [work.tile([128, NK], F32R, tag=f"k2{m}", name=f"k2{m}") for m in range(MB)]
            kd_sb = [work.tile([128, NKE], F32R, tag=f"kd{m}", name=f"kd{m}") for m in range(MB)]
            s_acc = work.tile([128, TT * NH], F32, tag="sacc")
            s_cols = work.tile([128, TT], F32, tag="scols")
            sflat = work.tile([1, T], F32R, tag="sflat")

            # ---- P1: q projection (streamed over 512-token chunks) ----
            with nc.named_scope("q_proj"):
                for t4 in range(TT4):
                    xq_t = []
                    for kc in range(KC):
                        xt = xqs.tile([128, 512], F32R, tag=f"xq{kc}")
                        nc.sync.dma_start(
                            out=xt[:],
                            in_=xq_in[kc * 128:(kc + 1) * 128,
                                      t4 * 512:(t4 + 1) * 512])
                        xq_t.append(xt)
                    for m in range(MB):
                        pq = psA.tile([128, 1024], F32, tag="sbank")
                        for kc in range(KC):
                            nc.tensor.matmul(
                                pq[:, 0:512],
                                wq_t[kc][:, m * 128:(m + 1) * 128],
                                xq_t[kc][:],
                                start=(kc == 0), stop=(kc == KC - 1))
                        if m % 2 == 0:
                            nc.scalar.copy(
                                q_sb[m][:, t4 * 512:(t4 + 1) * 512], pq[:, 0:512])
                        else:
                            nc.vector.tensor_copy(
                                q_sb[m][:, t4 * 512:(t4 + 1) * 512], pq[:, 0:512])

            # ---- conv inputs (needed from P2 on; DMAs issued after q's) ----
            xce_t, xcd_t, wksr_t = {}, {}, {}
            for e in range(4):
                for kc in range(KC):
                    t_ = xdat.tile([128, C], F32R, tag=f"wksr{e}_{kc}",
                                   name=f"wksr{e}_{kc}")
                    weng = nc.scalar if (e % 2 == 0) else nc.gpsimd
                    weng.dma_start(
                        out=t_[:], in_=wksr_in[e, kc * 128:(kc + 1) * 128, :])
                    wksr_t[(e, kc)] = t_
                    a = xdat.tile([128, NKE], F32R, tag=f"xce{e}_{kc}",
                                  name=f"xce{e}_{kc}")
                    nc.scalar.dma_start(
                        out=a[:], in_=xce_in[e, kc * 128:(kc + 1) * 128, :])
                    xce_t[(e, kc)] = a
                    d = xdat.tile([128, NKE], F32R, tag=f"xcd{e}_{kc}",
                                  name=f"xcd{e}_{kc}")
                    nc.gpsimd.dma_start(
                        out=d[:], in_=xcd_in[e, kc * 128:(kc + 1) * 128, :])
                    xcd_t[(e, kc)] = d

            # ---- P2: k_even / k_diff (2 blocks at a time, (e,kc) outer
            # so matmuls start as soon as the first conv chunks arrive) ----
            with nc.named_scope("k_proj"):
                for mh in range(2):
                    pks = [psA.tile([128, 1024], F32, tag="sbank",
                                    name=f"pk{mh}_{i}") for i in range(2)]
                    first = True
                    for e in range(4):
                        for kc in range(KC):
                            for i in range(2):
                                m = mh * 2 + i
                                nc.tensor.matmul(
                                    pks[i][:, 0:512],
                                    wksr_t[(e, kc)][:, m * 128:(m + 1) * 128],
                                    xce_t[(e, kc)][:],
                                    start=first, stop=(e == 3 and kc == KC - 1))
                                nc.tensor.matmul(
                                    pks[i][:, 512:1024],
                                    wksr_t[(e, kc)][:, m * 128:(m + 1) * 128],
                                    xcd_t[(e, kc)][:],
                                    start=first, stop=(e == 3 and kc == KC - 1))
                            first = False
                    for i in range(2):
                        m = mh * 2 + i
                        nc.scalar.activation(
                            k2_sb[m][:, 0:512], pks[i][:, 0:512], ACTF.Identity,
                            bias=ck_t[:, m:m + 1], scale=1.0)
                        nc.scalar.copy(kd_sb[m][:], pks[i][:, 512:1024])
                        nc.vector.tensor_add(
                            k2_sb[m][:, 512:1024], k2_sb[m][:, 0:512],
                            kd_sb[m][:])

            # ---- P3: scores + pair-max cascade + reduce ----
            # S slots (a-scores, later a+relu(D)) live psA; D slots live psB.
            def outer_half(hh):
                with nc.named_scope("outer"):
                    sl = slice(hh * (TT // 2), (hh + 1) * (TT // 2))
                    nc.sync.dma_start(out=sbounce[:, sl], in_=s_cols[:, sl])
                    nc.gpsimd.dma_start(
                        out=sflat[0:1, hh * (T // 2):(hh + 1) * (T // 2)],
                        in_=sbounce[:, sl].rearrange("p t -> () t p"))
                    pbc = psA.tile([128, 1024], F32, tag="sbank",
                                   name=f"pbc{hh}")
                    for t2 in range(2):
                        nc.tensor.matmul(
                            pbc[:, t2 * 512:(t2 + 1) * 512], ones[:],
                            sflat[0:1,
                                  hh * (T // 2) + t2 * 512:
                                  hh * (T // 2) + (t2 + 1) * 512],
                            start=True, stop=True)
                    for m in range(MB):
                        osb = opool.tile([128, T // 2], F32, tag="osb",
                                         name=f"osb{hh}_{m}")
                        nc.scalar.activation(
                            osb[:], pbc[:], ACTF.Identity,
                            bias=bb_t[:, m:m + 1], scale=pv_t[:, m:m + 1])
                        nc.sync.dma_start(
                            out=out_ext[m * 128:(m + 1) * 128,
                                        hh * (T // 2):(hh + 1) * (T // 2)],
                            in_=osb[:])

            # Software-pipelined emission, all-trick. Per step g:
            #   relus of g-1 (ScalarE, early so PE's D-mms of g+1 are safe),
            #   S/D matmuls of g (PE; D slots are per-head 1-bank tiles),
            #   identity-accumulates of g-2 (PE; relus finished a step ago),
            #   reduce of g-2 (VectorE).
            # S-slots: 3x2 banks; D-slots: 2x1 bank -> 8 PSUM banks.
            NG = TT * MB
            state = {}

            def emit_relus(g):
                pS, pDa, pDb, rr = state[g]
                ra = rpool.tile([128, 512], F32R, tag="r", name=f"ra{g}")
                rb = rpool.tile([128, 512], F32R, tag="r", name=f"rb{g}")
                nc.scalar.activation(ra[:], pDa[:], ACTF.Relu)
                nc.scalar.activation(rb[:], pDb[:], ACTF.Relu)
                state[g] = (pS, pDa, pDb, (ra, rb))

            def emit_front(g):
                tt, m = divmod(g, MB)
                qs = q_sb[m]
                tsl = slice(tt * 128, (tt + 1) * 128)
                pS = psA.tile([128, 1024], F32, tag="sbank", name=f"pS{g}")
                pDa = psD.tile([128, 512], F32, tag="dbank", name=f"pDa{g}")
                pDb = psD.tile([128, 512], F32, tag="dbank", name=f"pDb{g}")
                nc.tensor.matmul(pS[:, 0:512], qs[0:64, tsl],
                                 k2_sb[m][0:64, 0:512], start=True,
                                 stop=True, tile_position=(0, 0))
                nc.tensor.matmul(pS[:, 512:1024], qs[64:128, tsl],
                                 k2_sb[m][64:128, 0:512], start=True,
                                 stop=True, tile_position=(64, 0))
                nc.tensor.matmul(pDa[:], qs[0:64, tsl],
                                 kd_sb[m][0:64, :], start=True,
                                 stop=True, tile_position=(0, 0))
                nc.tensor.matmul(pDb[:], qs[64:128, tsl],
                                 kd_sb[m][64:128, :], start=True,
                                 stop=True, tile_position=(64, 0))
                state[g] = (pS, pDa, pDb, None)

            def emit_iadd(g):
                # one identity-accumulate adds both heads' relu corrections
                # (rhs spans 1024 columns -> two PSUM banks)
                pS, pDa, pDb, (ra, rb) = state[g]
                nc.tensor.matmul(pS[:, 0:512], ident[:], ra[:],
                                 start=False, stop=True)
                nc.tensor.matmul(pS[:, 512:1024], ident[:], rb[:],
                                 start=False, stop=True)

            def emit_back(g):
                tt, m = divmod(g, MB)
                pS = state.pop(g)[0]
                cols = slice(tt * NH + 2 * m, tt * NH + 2 * m + 2)
                nc.vector.reduce_max(
                    s_acc[:, cols],
                    pS[:].rearrange("p (a b) -> p a b", a=2), axis=AX.X)
                if m == MB - 1:
                    nc.vector.reduce_sum(
                        s_cols[:, tt:tt + 1],
                        s_acc[:, tt * NH:(tt + 1) * NH], axis=AX.X)
                    if tt == 9:
                        outer_half(0)
                    elif tt == TT - 1:
                        outer_half(1)

            with nc.named_scope("scores"):
                for g in range(NG + 2):
                    if g < NG:
                        emit_front(g)
                        emit_relus(g)
                    if g >= 2:
                        emit_iadd(g - 2)
                        emit_back(g - 2)

    _split_multi_waits(nc)
    return nc


# ---------------------------------------------------------------------------
# host side
# ---------------------------------------------------------------------------
def _prep_host(x, Wq, Wk, Wsr, bsr, bn_gamma, bn_beta, bn_mean, bn_var,
               Wproj, bproj):
    f8 = np.float64
    scale = HD ** -0.5
    g = bn_gamma.astype(f8) / np.sqrt(bn_var.astype(f8) + EPS)
    A = Wk.astype(f8) * g[None, :]
    ck = A @ bsr.astype(f8) + Wk.astype(f8) @ (
        bn_beta.astype(f8) - bn_mean.astype(f8) * g)
    wksr = np.stack([
        (A @ Wsr[:, :, e // 2, e % 2].astype(f8)).T for e in range(4)
    ]).astype(np.float32)                              # (4, C_in, C_out)
    wqT = (Wq.astype(f8) * scale).T.astype(np.float32)  # (C_in, C_out)

    x4 = x.reshape(B, C, HW_, HW_)
    xce = np.empty((B, 4, C, NKE), np.float32)
    xcd = np.empty((B, 4, C, NKE), np.float32)
    for e in range(4):
        di, dj = e // 2, e % 2
        even = x4[:, :, di::2, dj::4].reshape(B, C, NKE)
        odd = x4[:, :, di::2, dj + 2::4].reshape(B, C, NKE)
        xce[:, e] = even
        xcd[:, e] = odd - even

    v = x.astype(f8).mean(axis=2)                       # (B, C)
    pv = (Wproj.astype(f8) @ v.T).T.astype(np.float32)  # (B, C)

    ck_t = ck.astype(np.float32).reshape(MB, 128).T.copy()    # (128, MB)
    bb_t = bproj.astype(np.float32).reshape(MB, 128).T.copy()
    pv_t = [pv[b].reshape(MB, 128).T.copy() for b in range(B)]
    return wqT, wksr, ck_t, bb_t, pv_t, xce, xcd


def kernel(x, y, Wq, Wk, Wsr, bsr, bn_gamma, bn_beta, bn_mean, bn_var,
           Wproj, bproj, H, W):
    x = np.asarray(x, np.float32)
    wqT, wksr, ck_t, bb_t, pv_t, xce, xcd = _prep_host(
        x, np.asarray(Wq, np.float32), np.asarray(Wk, np.float32),
        np.asarray(Wsr, np.float32), np.asarray(bsr, np.float32),
        np.asarray(bn_gamma, np.float32), np.asarray(bn_beta, np.float32),
        np.asarray(bn_mean, np.float32), np.asarray(bn_var, np.float32),
        np.asarray(Wproj, np.float32), np.asarray(bproj, np.float32))

    _install_fixes()
    _install_ntff_hook()
    from concourse.bass_utils import run_bass_kernel_spmd

    if "nc" not in _cache:
        _cache["nc"] = _build_program()
    nc = _cache["nc"]

    ident = np.eye(128, dtype=np.float32)
    ones = np.ones((1, 128), np.float32)
    in_maps = []
    for core in range(NCORES):
        b, half = core // 2, core % 2
        in_maps.append({
            "xq": np.ascontiguousarray(x[b][:, half * T:(half + 1) * T]),
            "xce": xce[b], "xcd": xcd[b],
            "wq": wqT, "wksr": wksr,
            "ck": ck_t, "pv": pv_t[b], "bb": bb_t,
            "ident": ident, "ones": ones,
        })

    trace = os.environ.get("BASS_KERNEL_TRACE", "0") == "1"
    res = run_bass_kernel_spmd(nc, in_maps, list(range(NCORES)), trace=trace)
    if trace:
        print(f"HW exec time: {res.exec_time_ns} ns")
        _cache["last_exec_time_ns"] = res.exec_time_ns
        _cache["last_trace"] = res.instructions_and_trace

    out = np.empty((B, C, N), np.float32)
    for core in range(NCORES):
        b, half = core // 2, core % 2
        out[b][:, half * T:(half + 1) * T] = res.results[core]["out"]
    return out
